# revision 12
# baseline (speedup 1.0000x reference)
"""Trainium2 Bass kernel for nn_CGEBlock (Clifford Group Equivariant block, Cl(3,0)).

Strategy: pure data-parallel over the batch dim (8 cores x 16384 points).
Internally blades are kept in *mask order* (blade index == bitmask of basis
vectors), which makes the geometric-product pairing (i, k=i^j) expressible as
nested +-stride access patterns on the Vector engine. The final result is
permuted back to the reference blade order on the host.

Layout on device: "natural" — batch on partitions (tiles of 128 points),
features*blades on the free dim. Matmuls run on PE with the activation tile
transposed via the tensor engine; gates/norms use ACT; products use DVE.
"""

import sys

for p in ("/opt/trn_rl_repo",):
    if p not in sys.path:
        sys.path.insert(0, p)

import numpy as np

import concourse.bass as bass
import concourse.bacc as bacc
import concourse.mybir as mybir
import concourse.tile as tile
from concourse.bass_utils import run_bass_kernel_spmd
from concourse.masks import make_identity

EPS = 1e-6
N_CORES = 8
B_TOTAL = 131072
B_PC = B_TOTAL // N_CORES  # 16384
FIN = 16
FOUT = 32

# blade index (reference order) -> bitmask; also its own inverse permutation
MASKS = [0, 1, 2, 4, 3, 5, 6, 7]
GRADE_IDX = [0, 1, 1, 1, 2, 2, 2, 3]
PC = [bin(m).count("1") for m in range(8)]  # grade of a mask
# mask positions per grade (for strided slicing of mask-ordered 8-blocks)
GPOS = {0: [0], 1: [1, 2, 4], 2: [3, 5, 6], 3: [7]}

F32 = mybir.dt.float32
AX = mybir.AxisListType
ALU = mybir.AluOpType
AF = mybir.ActivationFunctionType


def _cayley_sign(a, b):
    s, aa = 0, a >> 1
    while aa:
        s += bin(aa & b).count("1")
        aa >>= 1
    return -1.0 if (s & 1) else 1.0


def build_consts(w1, b1, a_relu, b_relu, wl, bl, wr, a_norm, gp_w, a_ln):
    """Host-side constant matrices (all mask-ordered on the feature axes)."""
    c = {}
    isq2 = 1.0 / np.sqrt(2.0)

    # W1big [128=(m,i_idx), 256=(n,jm)] : h = x @ W1big (+b1 on mask-0 blade)
    W1 = np.zeros((128, 256), np.float32)
    for m in range(FIN):
        for ii in range(8):
            jm = MASKS[ii]
            for n in range(FOUT):
                W1[m * 8 + ii, n * 8 + jm] = w1[n, m, GRADE_IDX[ii]]
    c["W1big"] = W1

    # Wr halves [128=(n,im)half, 256=(n2,jm)]
    WrA = np.zeros((128, 256), np.float32)
    WrB = np.zeros((128, 256), np.float32)
    WlA = np.zeros((128, 256), np.float32)
    WlB = np.zeros((128, 256), np.float32)
    for n in range(FOUT):
        half, loc = (WrA, n) if n < 16 else (WrB, n - 16)
        halfl = WlA if n < 16 else WlB
        for im in range(8):
            g = PC[im]
            for n2 in range(FOUT):
                half[loc * 8 + im, n2 * 8 + im] = wr[n2, n, g]
                halfl[loc * 8 + im, n2 * 8 + im] = wl[n2, n, g] * a_ln[n2] * isq2
    c["WrA"], c["WrB"], c["WlA"], c["WlB"] = WrA, WrB, WlA, WlB

    rep = lambda v: np.repeat(v[None, :].astype(np.float32), 128, 0)
    c["b1r"] = rep(b1)
    c["blr"] = rep(bl * a_ln * isq2)
    c["invalnr"] = rep(1.0 / a_ln)

    # gate / norm rows, g-major layout: col = g*32 + n
    c["arelur"] = rep(a_relu.T.reshape(-1))
    c["brelur"] = rep(b_relu.T.reshape(-1))
    sig = 1.0 / (1.0 + np.exp(-a_norm))
    c["signr"] = rep(sig.T.reshape(-1))
    c["bias2r"] = rep((1.0 - sig + EPS).T.reshape(-1))

    # wrows [128, 8*256]: coeff for product term (h-blade i) at out col (cn,jm)
    W = np.zeros((8, 256), np.float32)
    for i in range(8):
        for jm in range(8):
            s = _cayley_sign(i, jm)
            gw = gp_w[:, PC[i], PC[jm], PC[i ^ jm]]  # [C]
            for cn in range(FOUT):
                W[i, cn * 8 + jm] = s * gw[cn] * a_ln[cn] * isq2
    c["wrows"] = np.repeat(W.reshape(1, -1), 128, 0).astype(np.float32)
    return c


CONST_SHAPES = {
    "W1big": (128, 256),
    "WrA": (128, 256),
    "WrB": (128, 256),
    "WlA": (128, 256),
    "WlB": (128, 256),
    "b1r": (128, 32),
    "blr": (128, 32),
    "invalnr": (128, 32),
    "arelur": (128, 128),
    "brelur": (128, 128),
    "signr": (128, 128),
    "bias2r": (128, 128),
    "wrows": (128, 2048),
}


def _ap(t, off, levels):
    """Custom free-dim AP on tile t: keep partition level, replace free levels."""
    a = t[:]
    return bass.AP(tensor=a.tensor, offset=a.offset + off, ap=[list(a.ap[0])] + levels)


def _xor_levels(i, cstep=8):
    """Nested levels reading index c*cstep + (i ^ j) as (c, j2, j1, j0)."""
    lv = [[cstep, 32]]
    for b in (4, 2, 1):
        lv.append([-b if (i & b) else b, 2])
    return lv


def build_program(b_pc=B_PC):
    nc = bacc.Bacc()
    x_d = nc.dram_tensor("x", [b_pc, 128], F32, kind="ExternalInput")
    out_d = nc.dram_tensor("out", [b_pc, 256], F32, kind="ExternalOutput")
    cd = {
        k: nc.dram_tensor(k, list(s), F32, kind="ExternalInput")
        for k, s in CONST_SHAPES.items()
    }

    n_grp = b_pc // 512
    xv = x_d[:].rearrange("(g s p) f -> p g s f", s=4, p=128)
    ov = out_d[:].rearrange("(g s p) f -> p g s f", s=4, p=128)

    with tile.TileContext(nc) as tc:
        with (
            tc.tile_pool(name="consts", bufs=1) as consts,
            tc.tile_pool(name="io", bufs=3) as io,
            tc.tile_pool(name="work", bufs=2) as work,
            tc.tile_pool(name="ps", bufs=1, space="PSUM") as ps,
        ):
            C = {}
            for k, s in CONST_SHAPES.items():
                C[k] = consts.tile(list(s), F32, name=k, tag=k)
                nc.sync.dma_start(out=C[k], in_=cd[k][:])
            ident = consts.tile([128, 128], F32)
            make_identity(nc, ident)

            for g in range(n_grp):
                xq = io.tile([128, 4, 128], F32)
                nc.sync.dma_start(out=xq, in_=xv[:, g, :, :])
                outq = io.tile([128, 4, 256], F32)

                for s in range(4):
                    x_nat = xq[:, s, :]
                    # ---- h = mvlinear1(x) ----
                    xT_ps = ps.tile([128, 128], F32, bufs=1, tag="xT")
                    nc.tensor.transpose(xT_ps[:], x_nat, ident[:])
                    xT = work.tile([128, 128], F32, tag="xT_sb")
                    nc.scalar.activation(xT[:], xT_ps[:], AF.Copy)
                    h_ps = ps.tile([128, 256], F32, bufs=2, tag="h_ps")
                    nc.tensor.matmul(h_ps[:], lhsT=xT[:], rhs=C["W1big"][:],
                                     start=True, stop=True)
                    h_c = h_ps[:].rearrange("p (c j) -> p c j", j=8)
                    nc.vector.tensor_add(h_c[:, :, 0], h_c[:, :, 0], C["b1r"][:])

                    h = work.tile([128, 256], F32, tag="h")
                    h2 = work.tile([128, 256], F32, tag="h2")
                    nc.scalar.activation(h[:], h_ps[:], AF.Copy)
                    nc.scalar.activation(h2[:], h_ps[:], AF.Square)
                    hv = h[:].rearrange("p (c j) -> p c j", j=8)
                    h2v = h2[:].rearrange("p (c j) -> p c j", j=8)

                    # ---- MVReLU gates (invt g-major [g*32+n]) ----
                    invt = work.tile([128, 128], F32, tag="invt")
                    nc.vector.tensor_copy(invt[:, 0:32], hv[:, :, 0])
                    nc.vector.tensor_add(invt[:, 32:64], h2v[:, :, 1], h2v[:, :, 2])
                    nc.vector.tensor_add(invt[:, 32:64], invt[:, 32:64], h2v[:, :, 4])
                    nc.vector.tensor_add(invt[:, 64:96], h2v[:, :, 3], h2v[:, :, 5])
                    nc.vector.tensor_add(invt[:, 64:96], invt[:, 64:96], h2v[:, :, 6])
                    nc.vector.tensor_copy(invt[:, 96:128], h2v[:, :, 7])
                    gp = work.tile([128, 128], F32, tag="gp")
                    nc.vector.tensor_mul(gp[:], invt[:], C["arelur"][:])
                    nc.vector.tensor_add(gp[:], gp[:], C["brelur"][:])
                    nc.vector.tensor_scalar_max(gp[:], gp[:], 0.0)

                    hg = work.tile([128, 256], F32, tag="hg")
                    hgv = hg[:].rearrange("p (c j) -> p c j", j=8)
                    for jm in range(8):
                        gslc = gp[:, PC[jm] * 32:PC[jm] * 32 + 32]
                        nc.vector.tensor_mul(hgv[:, :, jm], hv[:, :, jm], gslc)

                    # ---- xr = mvlinear(hg, wr) ----
                    hgT_ps = ps.tile([128, 256], F32, bufs=1, tag="hgT")
                    nc.tensor.transpose(hgT_ps[:, 0:128], hg[:, 0:128], ident[:])
                    nc.tensor.transpose(hgT_ps[:, 128:256], hg[:, 128:256], ident[:])
                    hgT = work.tile([128, 256], F32, tag="hgT_sb")
                    nc.scalar.activation(hgT[:], hgT_ps[:], AF.Copy)
                    xr_ps = ps.tile([128, 256], F32, bufs=2, tag="xr_ps")
                    nc.tensor.matmul(xr_ps[:], lhsT=hgT[:, 0:128], rhs=C["WrA"][:],
                                     start=True, stop=False)
                    nc.tensor.matmul(xr_ps[:], lhsT=hgT[:, 128:256], rhs=C["WrB"][:],
                                     start=False, stop=True)
                    xr = work.tile([128, 256], F32, tag="xr")
                    xr2 = work.tile([128, 256], F32, tag="xr2")
                    nc.scalar.activation(xr[:], xr_ps[:], AF.Copy)
                    nc.scalar.activation(xr2[:], xr_ps[:], AF.Square)
                    xrv = xr[:].rearrange("p (c j) -> p c j", j=8)
                    xr2v = xr2[:].rearrange("p (c j) -> p c j", j=8)

                    # ---- steerable norms: qst (g-major), rden = 1/(sig*n+1-sig+eps)
                    qst = work.tile([128, 128], F32, tag="qst")
                    nc.vector.tensor_copy(qst[:, 0:32], xr2v[:, :, 0])
                    nc.vector.tensor_add(qst[:, 32:64], xr2v[:, :, 1], xr2v[:, :, 2])
                    nc.vector.tensor_add(qst[:, 32:64], qst[:, 32:64], xr2v[:, :, 4])
                    nc.vector.tensor_add(qst[:, 64:96], xr2v[:, :, 3], xr2v[:, :, 5])
                    nc.vector.tensor_add(qst[:, 64:96], qst[:, 64:96], xr2v[:, :, 6])
                    nc.vector.tensor_copy(qst[:, 96:128], xr2v[:, :, 7])
                    nt = work.tile([128, 128], F32, tag="nt")
                    nc.scalar.activation(nt[:], qst[:], AF.Sqrt)
                    dent = work.tile([128, 128], F32, tag="dent")
                    nc.vector.tensor_mul(dent[:], nt[:], C["signr"][:])
                    nc.vector.tensor_add(dent[:], dent[:], C["bias2r"][:])
                    rden = work.tile([128, 128], F32, tag="rden")
                    rsc = work.tile([128, 128], F32, tag="rsc")
                    nc.vector.reciprocal_approx_accurate(rden[:], dent[:], rsc[:])

                    xrn = work.tile([128, 256], F32, tag="xrn")
                    xrnv = xrn[:].rearrange("p (c j) -> p c j", j=8)
                    for jm in range(8):
                        rslc = rden[:, PC[jm] * 32:PC[jm] * 32 + 32]
                        nc.vector.tensor_mul(xrnv[:, :, jm], xrv[:, :, jm], rslc)

                    # ---- geometric product: V[c,j,i] = w_i(c,j)*xrn[c,i^j] ----
                    # DVE APs allow at most 3 free dims, so split each h-blade
                    # term over the j2 bit: iteration order (c, j1, j0).
                    V = work.tile([128, 2048], F32, tag="V")
                    for i in range(8):
                        s2 = -2 if (i & 2) else 2
                        s1 = -1 if (i & 1) else 1
                        for j2 in (0, 1):
                            k2 = (i ^ (j2 << 2)) & 4  # high bit of xrn index
                            out_ap = _ap(V, i + j2 * 32, [[64, 32], [16, 2], [8, 2]])
                            xr_ap = _ap(xrn, (i & 3) | k2, [[8, 32], [s2, 2], [s1, 2]])
                            w_ap = _ap(C["wrows"], i * 256 + j2 * 4,
                                       [[8, 32], [2, 2], [1, 2]])
                            nc.vector.tensor_tensor(out_ap, xr_ap, w_ap, ALU.mult)
                    P2 = work.tile([128, 2048], F32, tag="P2")
                    hg_ap = _ap(hg, 0, [[8, 32], [0, 8], [1, 8]])
                    v_ap = _ap(V, 0, [[64, 32], [8, 8], [1, 8]])
                    p2_ap = _ap(P2, 0, [[64, 32], [8, 8], [1, 8]])
                    nc.vector.tensor_tensor(p2_ap, hg_ap, v_ap, ALU.mult)
                    geo = work.tile([128, 256], F32, tag="geo")
                    nc.vector.tensor_reduce(
                        geo[:], P2[:].rearrange("p (f i) -> p f i", i=8),
                        axis=AX.X, op=ALU.add)

                    # ---- hl + geo, layernorm, output ----
                    hf_ps = ps.tile([128, 256], F32, bufs=2, tag="hf_ps")
                    nc.tensor.matmul(hf_ps[:], lhsT=hgT[:, 0:128], rhs=C["WlA"][:],
                                     start=True, stop=False)
                    nc.tensor.matmul(hf_ps[:], lhsT=hgT[:, 128:256], rhs=C["WlB"][:],
                                     start=False, stop=True)
                    hfc = hf_ps[:].rearrange("p (c j) -> p c j", j=8)
                    nc.vector.tensor_add(hfc[:, :, 0], hfc[:, :, 0], C["blr"][:])
                    hf = work.tile([128, 256], F32, tag="hf")
                    nc.vector.tensor_add(hf[:], hf_ps[:], geo[:])

                    hf2 = work.tile([128, 256], F32, tag="hf2")
                    nc.scalar.activation(hf2[:], hf[:], AF.Square)
                    s32 = work.tile([128, 32], F32, tag="s32")
                    nc.vector.tensor_reduce(
                        s32[:], hf2[:].rearrange("p (c j) -> p c j", j=8),
                        axis=AX.X, op=ALU.add)
                    cn = work.tile([128, 32], F32, tag="cn")
                    nc.scalar.activation(cn[:], s32[:], AF.Sqrt)
                    nc.vector.tensor_mul(cn[:], cn[:], C["invalnr"][:])
                    snrm = work.tile([128, 1], F32, tag="snrm")
                    nc.vector.tensor_reduce(
                        snrm[:], cn[:].unsqueeze(1),
                        axis=AX.X, op=ALU.add)
                    den = work.tile([128, 1], F32, tag="den")
                    nc.vector.tensor_scalar(den[:], snrm[:], 1.0 / 32.0, EPS,
                                            op0=ALU.mult, op1=ALU.add)
                    rr = work.tile([128, 1], F32, tag="rr")
                    nc.vector.reciprocal(rr[:], den[:])
                    nc.vector.tensor_scalar_mul(outq[:, s, :], hf[:], rr[:])

                nc.sync.dma_start(out=ov[:, g, :, :], in_=outq)
    nc.finalize()
    return nc


_PROG = {}
LAST_RESULT = None


def _get_program(b_pc):
    if b_pc not in _PROG:
        _PROG[b_pc] = build_program(b_pc)
    return _PROG[b_pc]


def kernel(**inputs):
    x = np.ascontiguousarray(np.asarray(inputs["x"], np.float32))
    consts = build_consts(
        np.asarray(inputs["w1"], np.float32), np.asarray(inputs["b1"], np.float32),
        np.asarray(inputs["a_relu"], np.float32), np.asarray(inputs["b_relu"], np.float32),
        np.asarray(inputs["wl"], np.float32), np.asarray(inputs["bl"], np.float32),
        np.asarray(inputs["wr"], np.float32), np.asarray(inputs["a_norm"], np.float32),
        np.asarray(inputs["gp_w"], np.float32), np.asarray(inputs["a_ln"], np.float32),
    )
    b_total = x.shape[0]
    b_pc = b_total // N_CORES
    nc = _get_program(b_pc)
    in_maps = []
    for c in range(N_CORES):
        m = {"x": x[c * b_pc:(c + 1) * b_pc].reshape(b_pc, 128)}
        m.update(consts)
        in_maps.append(m)
    import os
    trace = os.environ.get("KERNEL_TRACE", "0") == "1"
    res = run_bass_kernel_spmd(nc, in_maps, core_ids=list(range(N_CORES)),
                               trace=trace)
    global LAST_RESULT
    LAST_RESULT = res
    outs = [
        res.results[c]["out"].reshape(b_pc, FOUT, 8)[:, :, MASKS]
        for c in range(N_CORES)
    ]
    return np.ascontiguousarray(np.concatenate(outs, axis=0).astype(np.float32))


if __name__ == "__main__":
    # smoke test with random data against a numpy re-implementation
    rng = np.random.default_rng(0)
    print("building program...")
    build_program(512)
    print("ok")


# revision 26
# speedup vs baseline: 1.4176x; 1.4176x over previous
"""Trainium2 Bass kernel for nn_CGEBlock (Clifford Group Equivariant block, Cl(3,0)).

Strategy: pure data-parallel over the batch dim (8 cores x 16384 points).
Internally blades are kept in *mask order* (blade index == bitmask of basis
vectors), which makes the geometric-product pairing (i, k=i^j) expressible as
nested +-stride access patterns on the Vector engine. The final result is
permuted back to the reference blade order on the host.

Layout on device: "natural" — batch on partitions (tiles of 128 points),
features*blades on the free dim. Matmuls run on PE with the activation tile
transposed via the tensor engine; gates/norms use ACT; products use DVE.
"""

import sys

for p in ("/opt/trn_rl_repo",):
    if p not in sys.path:
        sys.path.insert(0, p)

import numpy as np

import concourse.bass as bass
import concourse.bacc as bacc
import concourse.mybir as mybir
import concourse.tile as tile
from concourse.bass_utils import run_bass_kernel_spmd
from concourse.masks import make_identity

EPS = 1e-6
N_CORES = 8
B_TOTAL = 131072
B_PC = B_TOTAL // N_CORES  # 16384
FIN = 16
FOUT = 32

# blade index (reference order) -> bitmask; also its own inverse permutation
MASKS = [0, 1, 2, 4, 3, 5, 6, 7]
GRADE_IDX = [0, 1, 1, 1, 2, 2, 2, 3]
PC = [bin(m).count("1") for m in range(8)]  # grade of a mask
# mask positions per grade (for strided slicing of mask-ordered 8-blocks)
GPOS = {0: [0], 1: [1, 2, 4], 2: [3, 5, 6], 3: [7]}

F32 = mybir.dt.float32
BF16 = mybir.dt.bfloat16
AX = mybir.AxisListType
ALU = mybir.AluOpType
AF = mybir.ActivationFunctionType

# contiguous mask-position runs sharing one grade: (grade, [positions])
GRUNS = [(0, 0, 1), (1, 1, 2), (2, 3, 1), (1, 4, 1), (2, 5, 2), (3, 7, 1)]


def _cayley_sign(a, b):
    s, aa = 0, a >> 1
    while aa:
        s += bin(aa & b).count("1")
        aa >>= 1
    return -1.0 if (s & 1) else 1.0


def build_consts(w1, b1, a_relu, b_relu, wl, bl, wr, a_norm, gp_w, a_ln):
    """Host-side constant matrices.

    Feature axes use *blade-major* layout: column index = jm*32 + n, where jm
    is the blade bitmask and n the channel. This keeps every Vector-engine op
    contiguous over 32-channel runs (strided access patterns run ~5x slower).
    """
    c = {}
    isq2 = 1.0 / np.sqrt(2.0)

    # W1big [128=(m,i_idx), 256=(jm,n)] : h = x @ W1big (+b1 on mask-0 blade)
    W1 = np.zeros((128, 256), np.float32)
    for m in range(FIN):
        for ii in range(8):
            jm = MASKS[ii]
            for n in range(FOUT):
                W1[m * 8 + ii, jm * 32 + n] = w1[n, m, GRADE_IDX[ii]]
    c["W1big"] = W1

    # Wr/Wl halves: rows (jm,n) blade-major halves, cols (jm',n') blade-major
    WrA = np.zeros((128, 256), np.float32)
    WrB = np.zeros((128, 256), np.float32)
    WlA = np.zeros((128, 256), np.float32)
    WlB = np.zeros((128, 256), np.float32)
    for jm in range(8):
        g = PC[jm]
        half, base = (WrA, jm * 32) if jm < 4 else (WrB, (jm - 4) * 32)
        halfl = WlA if jm < 4 else WlB
        for n in range(FOUT):
            for n2 in range(FOUT):
                half[base + n, jm * 32 + n2] = wr[n2, n, g]
                halfl[base + n, jm * 32 + n2] = wl[n2, n, g] * a_ln[n2] * isq2
    # merge Wr|Wl per K-half and split hi/lo in bf16 (removes weight rounding)
    import ml_dtypes
    for nm, a, b in (("A", WrA, WlA), ("B", WrB, WlB)):
        M = np.concatenate([a, b], axis=1)
        hi = M.astype(ml_dtypes.bfloat16).astype(np.float32)
        lo = M - hi
        c[f"WW{nm}_hi"] = hi
        c[f"WW{nm}_lo"] = lo

    rep = lambda v: np.repeat(v[None, :].astype(np.float32), 128, 0)
    c["b1r"] = rep(b1)
    c["blr"] = rep(bl * a_ln * isq2)
    c["invalnr"] = rep(1.0 / a_ln)

    # gate / norm rows, g-major layout: col = g*32 + n
    c["arelur"] = rep(a_relu.T.reshape(-1))
    c["brelur"] = rep(b_relu.T.reshape(-1))
    sig = 1.0 / (1.0 + np.exp(-a_norm))
    c["signr"] = rep(sig.T.reshape(-1))
    c["bias2r"] = rep((1.0 - sig + EPS).T.reshape(-1))

    # wrowsP [128, 8*256], layout (i, jm, n): coeff for h-blade i at (jm, n)
    W = np.zeros((8, 256), np.float32)
    for i in range(8):
        for jm in range(8):
            s = _cayley_sign(i, jm)
            gw = gp_w[:, PC[i], PC[jm], PC[i ^ jm]]  # [C]
            for cn in range(FOUT):
                W[i, jm * 32 + cn] = s * gw[cn] * a_ln[cn] * isq2
    c["wrows"] = np.repeat(W.reshape(1, -1), 128, 0).astype(np.float32)
    return c


CONST_SHAPES = {
    "W1big": ((128, 256), "f32"),
    "WWA_hi": ((128, 512), "bf16"),
    "WWA_lo": ((128, 512), "bf16"),
    "WWB_hi": ((128, 512), "bf16"),
    "WWB_lo": ((128, 512), "bf16"),
    "b1r": ((128, 32), "f32"),
    "blr": ((128, 32), "f32"),
    "invalnr": ((128, 32), "f32"),
    "arelur": ((128, 128), "f32"),
    "brelur": ((128, 128), "f32"),
    "signr": ((128, 128), "f32"),
    "bias2r": ((128, 128), "f32"),
    "wrows": ((128, 2048), "f32"),
}


def _ap(t, off, levels):
    """Custom free-dim AP on tile t: keep partition level, replace free levels."""
    a = t[:]
    return bass.AP(tensor=a.tensor, offset=a.offset + off, ap=[list(a.ap[0])] + levels)


def _xor_levels(i, cstep=8):
    """Nested levels reading index c*cstep + (i ^ j) as (c, j2, j1, j0)."""
    lv = [[cstep, 32]]
    for b in (4, 2, 1):
        lv.append([-b if (i & b) else b, 2])
    return lv


def build_program(b_pc=B_PC):
    nc = bacc.Bacc()
    x_d = nc.dram_tensor("x", [b_pc, 128], F32, kind="ExternalInput")
    out_d = nc.dram_tensor("out", [b_pc, 256], F32, kind="ExternalOutput")
    cd = {
        k: nc.dram_tensor(k, list(s), F32 if t == "f32" else BF16,
                          kind="ExternalInput")
        for k, (s, t) in CONST_SHAPES.items()
    }

    n_grp = b_pc // 512
    xv = x_d[:].rearrange("(g s p) f -> p g s f", s=4, p=128)
    ov = out_d[:].rearrange("(g s p) f -> p g s f", s=4, p=128)

    with tile.TileContext(nc) as tc:
        with (
            tc.tile_pool(name="consts", bufs=1) as consts,
            tc.tile_pool(name="io", bufs=3) as io,
            tc.tile_pool(name="work", bufs=2) as work,
            tc.tile_pool(name="ps", bufs=1, space="PSUM") as ps,
        ):
            C = {}
            for k, (s, t) in CONST_SHAPES.items():
                C[k] = consts.tile(list(s), F32 if t == "f32" else BF16,
                                   name=k, tag=k)
                nc.sync.dma_start(out=C[k], in_=cd[k][:])
            ident = consts.tile([128, 128], F32)
            make_identity(nc, ident)
            ident16 = consts.tile([128, 128], BF16)
            nc.vector.tensor_copy(ident16[:], ident[:])

            for g in range(n_grp):
                xq = io.tile([128, 4, 128], F32)
                nc.sync.dma_start(out=xq, in_=xv[:, g, :, :])
                outq = io.tile([128, 4, 256], F32)

                # ---- h = mvlinear1(x): per-subtile transpose + matmul ----
                h_ps = ps.tile([128, 4, 256], F32, bufs=1, tag="h_ps")
                for s in range(4):
                    xT_ps = ps.tile([128, 128], F32, bufs=1, tag="xT")
                    nc.tensor.transpose(xT_ps[:], xq[:, s, :], ident[:])
                    xT = work.tile([128, 128], F32, bufs=2, tag="xT_sb")
                    nc.scalar.activation(xT[:], xT_ps[:], AF.Copy)
                    nc.tensor.matmul(h_ps[:, s, :], lhsT=xT[:], rhs=C["W1big"][:],
                                     start=True, stop=True)
                nc.vector.tensor_tensor(
                    _ap(h_ps, 0, [[256, 4], [1, 32]]),
                    _ap(h_ps, 0, [[256, 4], [1, 32]]),
                    _ap(C["b1r"], 0, [[0, 4], [1, 32]]), ALU.add)
                h = work.tile([128, 4, 256], F32, bufs=2, tag="h")
                h2 = work.tile([128, 4, 256], F32, bufs=2, tag="h2")
                nc.scalar.activation(h[:], h_ps[:], AF.Copy)
                nc.scalar.activation(h2[:], h_ps[:], AF.Square)

                # ---- MVReLU gates, g-major [g*32+n], batched over subtiles
                invt = work.tile([128, 4, 128], F32, bufs=1, tag="invt")
                nc.vector.tensor_copy(invt[:, :, 0:32], h[:, :, 0:32])
                nc.vector.tensor_add(invt[:, :, 32:64], h2[:, :, 32:64], h2[:, :, 64:96])
                nc.vector.tensor_add(invt[:, :, 32:64], invt[:, :, 32:64], h2[:, :, 128:160])
                nc.vector.tensor_add(invt[:, :, 64:96], h2[:, :, 96:128], h2[:, :, 160:192])
                nc.vector.tensor_add(invt[:, :, 64:96], invt[:, :, 64:96], h2[:, :, 192:224])
                nc.vector.tensor_copy(invt[:, :, 96:128], h2[:, :, 224:256])
                gp = work.tile([128, 4, 128], F32, bufs=1, tag="gp")
                arl = _ap(C["arelur"], 0, [[0, 4], [1, 128]])
                brl = _ap(C["brelur"], 0, [[0, 4], [1, 128]])
                nc.gpsimd.tensor_tensor(gp[:], invt[:], arl, ALU.mult)
                nc.gpsimd.tensor_tensor(gp[:], gp[:], brl, ALU.add)
                nc.vector.tensor_scalar_max(gp[:], gp[:], 0.0)

                # gate-mul (blade-major j-runs), f32 out for hi/lo split
                hgF = work.tile([128, 4, 256], F32, bufs=1, tag="hgF")
                for grade, j0, ln in ((0, 0, 1), (1, 1, 2), (2, 3, 1),
                                      (1, 4, 1), (2, 5, 2), (3, 7, 1)):
                    o_ap = _ap(hgF, j0 * 32, [[256, 4], [32, ln], [1, 32]])
                    h_ap = _ap(h, j0 * 32, [[256, 4], [32, ln], [1, 32]])
                    g_ap = _ap(gp, grade * 32, [[128, 4], [0, ln], [1, 32]])
                    nc.vector.tensor_tensor(o_ap, h_ap, g_ap, ALU.mult)
                hg_hi = work.tile([128, 4, 256], BF16, bufs=2, tag="hg_hi")
                nc.vector.tensor_copy(hg_hi[:], hgF[:])
                hg_hiF = work.tile([128, 4, 256], F32, bufs=1, tag="hg_hiF")
                nc.scalar.activation(hg_hiF[:], hg_hi[:], AF.Copy)
                hg_lo = work.tile([128, 4, 256], BF16, bufs=2, tag="hg_lo")
                nc.vector.tensor_tensor(hg_lo[:], hgF[:], hg_hiF[:], ALU.subtract)

                # ---- transposes of hi/lo halves, then merged Wr|Wl matmuls
                hgT_ps = ps.tile([128, 4, 4, 128], BF16, bufs=1, tag="hgT")
                for s in range(4):
                    nc.tensor.transpose(hgT_ps[:, s, 0, :], hg_hi[:, s, 0:128], ident16[:])
                    nc.tensor.transpose(hgT_ps[:, s, 1, :], hg_hi[:, s, 128:256], ident16[:])
                    nc.tensor.transpose(hgT_ps[:, s, 2, :], hg_lo[:, s, 0:128], ident16[:])
                    nc.tensor.transpose(hgT_ps[:, s, 3, :], hg_lo[:, s, 128:256], ident16[:])
                hgTs = work.tile([128, 4, 4, 128], BF16, bufs=2, tag="hgTs")
                nc.scalar.activation(hgTs[:], hgT_ps[:], AF.Copy)

                xr = work.tile([128, 4, 256], F32, bufs=2, tag="xr")
                xr2 = work.tile([128, 4, 256], F32, bufs=2, tag="xr2")
                hl_sb = work.tile([128, 4, 256], F32, bufs=2, tag="hl_sb")
                hf = work.tile([128, 4, 256], F32, bufs=2, tag="hf")
                geo = work.tile([128, 4, 256], F32, bufs=2, tag="geo")
                for s in range(4):
                    xrhl_ps = ps.tile([128, 512], F32, bufs=2, tag="xrhl")
                    mms = [(0, "WWA_hi", True, False), (1, "WWB_hi", False, False),
                           (0, "WWA_lo", False, False), (1, "WWB_lo", False, False),
                           (2, "WWA_hi", False, False), (3, "WWB_hi", False, True)]
                    for piece, wname, st, sp in mms:
                        nc.tensor.matmul(xrhl_ps[:], lhsT=hgTs[:, s, piece, :],
                                         rhs=C[wname][:], start=st, stop=sp)
                    nc.scalar.activation(xr[:, s, :], xrhl_ps[:, 0:256], AF.Copy)
                    nc.scalar.activation(xr2[:, s, :], xrhl_ps[:, 0:256], AF.Square)
                    nc.scalar.activation(hl_sb[:, s, :], xrhl_ps[:, 256:512], AF.Copy)

                # ---- steerable norms (batched) ----
                qst = work.tile([128, 4, 128], F32, bufs=1, tag="qst")
                nc.vector.tensor_copy(qst[:, :, 0:32], xr2[:, :, 0:32])
                nc.vector.tensor_add(qst[:, :, 32:64], xr2[:, :, 32:64], xr2[:, :, 64:96])
                nc.vector.tensor_add(qst[:, :, 32:64], qst[:, :, 32:64], xr2[:, :, 128:160])
                nc.vector.tensor_add(qst[:, :, 64:96], xr2[:, :, 96:128], xr2[:, :, 160:192])
                nc.vector.tensor_add(qst[:, :, 64:96], qst[:, :, 64:96], xr2[:, :, 192:224])
                nc.vector.tensor_copy(qst[:, :, 96:128], xr2[:, :, 224:256])
                nt = work.tile([128, 4, 128], F32, bufs=1, tag="nt")
                nc.scalar.activation(nt[:], qst[:], AF.Sqrt)
                dent = work.tile([128, 4, 128], F32, bufs=1, tag="dent")
                sgr = _ap(C["signr"], 0, [[0, 4], [1, 128]])
                b2r = _ap(C["bias2r"], 0, [[0, 4], [1, 128]])
                nc.gpsimd.tensor_tensor(dent[:], nt[:], sgr, ALU.mult)
                nc.gpsimd.tensor_tensor(dent[:], dent[:], b2r, ALU.add)
                rden = work.tile([128, 4, 128], F32, bufs=2, tag="rden")
                rsc = work.tile([128, 4, 128], F32, bufs=1, tag="rsc")
                nc.vector.reciprocal_approx_accurate(rden[:], dent[:], rsc[:])
                xrn = work.tile([128, 4, 256], F32, bufs=2, tag="xrn")
                for grade, j0, ln in ((0, 0, 1), (1, 1, 2), (2, 3, 1),
                                      (1, 4, 1), (2, 5, 2), (3, 7, 1)):
                    o_ap = _ap(xrn, j0 * 32, [[256, 4], [32, ln], [1, 32]])
                    x_ap = _ap(xr, j0 * 32, [[256, 4], [32, ln], [1, 32]])
                    r_ap = _ap(rden, grade * 32, [[128, 4], [0, ln], [1, 32]])
                    nc.vector.tensor_tensor(o_ap, x_ap, r_ap, ALU.mult)

                # ---- geometric product ----
                P2g = work.tile([128, 4, 2048], BF16, bufs=1, tag="P2g")
                for s in range(4):
                    V = work.tile([128, 2048], BF16, bufs=2, tag="V")
                    for i in range(8):
                        s2 = -64 if (i & 2) else 64
                        s1 = -32 if (i & 1) else 32
                        if (i & 3) in (0, 3):
                            s4 = -128 if (i & 4) else 128
                            out_ap = _ap(V, i * 256, [[128, 2], [32, 4], [1, 32]])
                            xr_ap = _ap(xrn, s * 256 + i * 32,
                                        [[s4, 2], [s1 // abs(s1) * 32, 4], [1, 32]])
                            w_ap = _ap(C["wrows"], i * 256,
                                       [[128, 2], [32, 4], [1, 32]])
                            nc.vector.tensor_tensor(out_ap, xr_ap, w_ap, ALU.mult)
                        else:
                            for j2 in (0, 1):
                                k0 = ((i ^ (j2 << 2)) & 4) * 32 + (i & 3) * 32
                                out_ap = _ap(V, i * 256 + j2 * 128,
                                             [[64, 2], [32, 2], [1, 32]])
                                xr_ap = _ap(xrn, s * 256 + k0,
                                            [[s2, 2], [s1, 2], [1, 32]])
                                w_ap = _ap(C["wrows"], i * 256 + j2 * 128,
                                           [[64, 2], [32, 2], [1, 32]])
                                nc.vector.tensor_tensor(out_ap, xr_ap, w_ap, ALU.mult)
                    hg_ap = _ap(hg_hi, s * 256, [[32, 8], [0, 8], [1, 32]])
                    v_ap = _ap(V, 0, [[256, 8], [32, 8], [1, 32]])
                    p2_ap = _ap(P2g, s * 2048, [[256, 8], [32, 8], [1, 32]])
                    nc.vector.tensor_tensor(p2_ap, hg_ap, v_ap, ALU.mult)
                # tree-reduce over i, in place, batched over subtiles
                nc.vector.tensor_tensor(
                    _ap(P2g, 0, [[2048, 4], [1, 1024]]),
                    _ap(P2g, 0, [[2048, 4], [1, 1024]]),
                    _ap(P2g, 1024, [[2048, 4], [1, 1024]]), ALU.add)
                nc.vector.tensor_tensor(
                    _ap(P2g, 0, [[2048, 4], [1, 512]]),
                    _ap(P2g, 0, [[2048, 4], [1, 512]]),
                    _ap(P2g, 512, [[2048, 4], [1, 512]]), ALU.add)
                nc.vector.tensor_tensor(
                    _ap(geo, 0, [[256, 4], [1, 256]]),
                    _ap(P2g, 0, [[2048, 4], [1, 256]]),
                    _ap(P2g, 256, [[2048, 4], [1, 256]]), ALU.add)

                nc.vector.tensor_tensor(
                    _ap(hl_sb, 0, [[256, 4], [1, 32]]),
                    _ap(hl_sb, 0, [[256, 4], [1, 32]]),
                    _ap(C["blr"], 0, [[0, 4], [1, 32]]), ALU.add)
                nc.vector.tensor_add(hf[:], hl_sb[:], geo[:])

                # ---- layernorm + output ----
                hf2 = work.tile([128, 4, 256], F32, bufs=2, tag="hf2")
                nc.scalar.activation(hf2[:], hf[:], AF.Square)
                u1 = work.tile([128, 4, 128], F32, bufs=1, tag="u1")
                nc.vector.tensor_add(u1[:], hf2[:, :, 0:128], hf2[:, :, 128:256])
                u2 = work.tile([128, 4, 64], F32, bufs=1, tag="u2")
                nc.vector.tensor_add(u2[:], u1[:, :, 0:64], u1[:, :, 64:128])
                s32 = work.tile([128, 4, 32], F32, bufs=1, tag="s32")
                nc.vector.tensor_add(s32[:], u2[:, :, 0:32], u2[:, :, 32:64])
                cn = work.tile([128, 4, 32], F32, bufs=1, tag="cn")
                nc.scalar.activation(cn[:], s32[:], AF.Sqrt)
                ivr = _ap(C["invalnr"], 0, [[0, 4], [1, 32]])
                nc.vector.tensor_tensor(cn[:], cn[:], ivr, ALU.mult)
                snrm = work.tile([128, 4], F32, bufs=1, tag="snrm")
                nc.vector.tensor_reduce(snrm[:].unsqueeze(2), cn[:],
                                        axis=AX.X, op=ALU.add)
                den = work.tile([128, 4], F32, bufs=1, tag="den")
                nc.vector.tensor_scalar(den[:], snrm[:], 1.0 / 32.0, EPS,
                                        op0=ALU.mult, op1=ALU.add)
                rr = work.tile([128, 4], F32, bufs=2, tag="rr")
                nc.vector.reciprocal(rr[:], den[:])
                for s in range(4):
                    nc.scalar.activation(outq[:, s, :], hf[:, s, :], AF.Identity,
                                         scale=rr[:, s:s + 1])

                nc.sync.dma_start(out=ov[:, g, :, :], in_=outq)
    nc.finalize()
    return nc


_PROG = {}
LAST_RESULT = None


def _get_program(b_pc):
    if b_pc not in _PROG:
        _PROG[b_pc] = build_program(b_pc)
    return _PROG[b_pc]


def kernel(**inputs):
    x = np.ascontiguousarray(np.asarray(inputs["x"], np.float32))
    consts = build_consts(
        np.asarray(inputs["w1"], np.float32), np.asarray(inputs["b1"], np.float32),
        np.asarray(inputs["a_relu"], np.float32), np.asarray(inputs["b_relu"], np.float32),
        np.asarray(inputs["wl"], np.float32), np.asarray(inputs["bl"], np.float32),
        np.asarray(inputs["wr"], np.float32), np.asarray(inputs["a_norm"], np.float32),
        np.asarray(inputs["gp_w"], np.float32), np.asarray(inputs["a_ln"], np.float32),
    )
    import ml_dtypes
    for k, (_, t) in CONST_SHAPES.items():
        if t == "bf16":
            consts[k] = consts[k].astype(ml_dtypes.bfloat16)
    b_total = x.shape[0]
    b_pc = b_total // N_CORES
    nc = _get_program(b_pc)
    in_maps = []
    for c in range(N_CORES):
        m = {"x": x[c * b_pc:(c + 1) * b_pc].reshape(b_pc, 128)}
        m.update(consts)
        in_maps.append(m)
    import os
    trace = os.environ.get("KERNEL_TRACE", "0") == "1"
    res = run_bass_kernel_spmd(nc, in_maps, core_ids=list(range(N_CORES)),
                               trace=trace)
    global LAST_RESULT
    LAST_RESULT = res
    outs = [
        res.results[c]["out"].reshape(b_pc, 8, FOUT)[:, MASKS, :].transpose(0, 2, 1)
        for c in range(N_CORES)
    ]
    return np.ascontiguousarray(np.concatenate(outs, axis=0).astype(np.float32))


if __name__ == "__main__":
    # smoke test with random data against a numpy re-implementation
    rng = np.random.default_rng(0)
    print("building program...")
    build_program(512)
    print("ok")


# revision 27
# speedup vs baseline: 1.4473x; 1.0209x over previous
"""Trainium2 Bass kernel for nn_CGEBlock (Clifford Group Equivariant block, Cl(3,0)).

Strategy: pure data-parallel over the batch dim (8 cores x 16384 points).
Internally blades are kept in *mask order* (blade index == bitmask of basis
vectors), which makes the geometric-product pairing (i, k=i^j) expressible as
nested +-stride access patterns on the Vector engine. The final result is
permuted back to the reference blade order on the host.

Layout on device: "natural" — batch on partitions (tiles of 128 points),
features*blades on the free dim. Matmuls run on PE with the activation tile
transposed via the tensor engine; gates/norms use ACT; products use DVE.
"""

import sys

for p in ("/opt/trn_rl_repo",):
    if p not in sys.path:
        sys.path.insert(0, p)

import numpy as np

import concourse.bass as bass
import concourse.bacc as bacc
import concourse.mybir as mybir
import concourse.tile as tile
from concourse.bass_utils import run_bass_kernel_spmd
from concourse.masks import make_identity

EPS = 1e-6
N_CORES = 8
B_TOTAL = 131072
B_PC = B_TOTAL // N_CORES  # 16384
FIN = 16
FOUT = 32

# blade index (reference order) -> bitmask; also its own inverse permutation
MASKS = [0, 1, 2, 4, 3, 5, 6, 7]
GRADE_IDX = [0, 1, 1, 1, 2, 2, 2, 3]
PC = [bin(m).count("1") for m in range(8)]  # grade of a mask
# mask positions per grade (for strided slicing of mask-ordered 8-blocks)
GPOS = {0: [0], 1: [1, 2, 4], 2: [3, 5, 6], 3: [7]}

F32 = mybir.dt.float32
BF16 = mybir.dt.bfloat16
AX = mybir.AxisListType
ALU = mybir.AluOpType
AF = mybir.ActivationFunctionType

# contiguous mask-position runs sharing one grade: (grade, [positions])
GRUNS = [(0, 0, 1), (1, 1, 2), (2, 3, 1), (1, 4, 1), (2, 5, 2), (3, 7, 1)]


def _cayley_sign(a, b):
    s, aa = 0, a >> 1
    while aa:
        s += bin(aa & b).count("1")
        aa >>= 1
    return -1.0 if (s & 1) else 1.0


def build_consts(w1, b1, a_relu, b_relu, wl, bl, wr, a_norm, gp_w, a_ln):
    """Host-side constant matrices.

    Feature axes use *blade-major* layout: column index = jm*32 + n, where jm
    is the blade bitmask and n the channel. This keeps every Vector-engine op
    contiguous over 32-channel runs (strided access patterns run ~5x slower).
    """
    c = {}
    isq2 = 1.0 / np.sqrt(2.0)

    # W1big [128=(m,i_idx), 256=(jm,n)] : h = x @ W1big (+b1 on mask-0 blade)
    W1 = np.zeros((128, 256), np.float32)
    for m in range(FIN):
        for ii in range(8):
            jm = MASKS[ii]
            for n in range(FOUT):
                W1[m * 8 + ii, jm * 32 + n] = w1[n, m, GRADE_IDX[ii]]
    c["W1big"] = W1

    # Wr/Wl halves: rows (jm,n) blade-major halves, cols (jm',n') blade-major
    WrA = np.zeros((128, 256), np.float32)
    WrB = np.zeros((128, 256), np.float32)
    WlA = np.zeros((128, 256), np.float32)
    WlB = np.zeros((128, 256), np.float32)
    for jm in range(8):
        g = PC[jm]
        half, base = (WrA, jm * 32) if jm < 4 else (WrB, (jm - 4) * 32)
        halfl = WlA if jm < 4 else WlB
        for n in range(FOUT):
            for n2 in range(FOUT):
                half[base + n, jm * 32 + n2] = wr[n2, n, g]
                halfl[base + n, jm * 32 + n2] = wl[n2, n, g] * a_ln[n2] * isq2
    # merge Wr|Wl per K-half and split hi/lo in bf16 (removes weight rounding)
    import ml_dtypes
    for nm, a, b in (("A", WrA, WlA), ("B", WrB, WlB)):
        M = np.concatenate([a, b], axis=1)
        hi = M.astype(ml_dtypes.bfloat16).astype(np.float32)
        lo = M - hi
        c[f"WW{nm}_hi"] = hi
        c[f"WW{nm}_lo"] = lo

    rep = lambda v: np.repeat(v[None, :].astype(np.float32), 128, 0)
    c["b1r"] = rep(b1)
    c["blr"] = rep(bl * a_ln * isq2)
    c["invalnr"] = rep(1.0 / a_ln)

    # gate / norm rows, g-major layout: col = g*32 + n
    c["arelur"] = rep(a_relu.T.reshape(-1))
    c["brelur"] = rep(b_relu.T.reshape(-1))
    sig = 1.0 / (1.0 + np.exp(-a_norm))
    c["signr"] = rep(sig.T.reshape(-1))
    c["bias2r"] = rep((1.0 - sig + EPS).T.reshape(-1))

    # wrowsP [128, 8*256], layout (i, jm, n): coeff for h-blade i at (jm, n)
    W = np.zeros((8, 256), np.float32)
    for i in range(8):
        for jm in range(8):
            s = _cayley_sign(i, jm)
            gw = gp_w[:, PC[i], PC[jm], PC[i ^ jm]]  # [C]
            for cn in range(FOUT):
                W[i, jm * 32 + cn] = s * gw[cn] * a_ln[cn] * isq2
    c["wrows"] = np.repeat(W.reshape(1, -1), 128, 0).astype(np.float32)
    return c


CONST_SHAPES = {
    "W1big": ((128, 256), "f32"),
    "WWA_hi": ((128, 512), "bf16"),
    "WWA_lo": ((128, 512), "bf16"),
    "WWB_hi": ((128, 512), "bf16"),
    "WWB_lo": ((128, 512), "bf16"),
    "b1r": ((128, 32), "f32"),
    "blr": ((128, 32), "f32"),
    "invalnr": ((128, 32), "f32"),
    "arelur": ((128, 128), "f32"),
    "brelur": ((128, 128), "f32"),
    "signr": ((128, 128), "f32"),
    "bias2r": ((128, 128), "f32"),
    "wrows": ((128, 2048), "bf16"),
}


def _ap(t, off, levels):
    """Custom free-dim AP on tile t: keep partition level, replace free levels."""
    a = t[:]
    return bass.AP(tensor=a.tensor, offset=a.offset + off, ap=[list(a.ap[0])] + levels)


def _xor_levels(i, cstep=8):
    """Nested levels reading index c*cstep + (i ^ j) as (c, j2, j1, j0)."""
    lv = [[cstep, 32]]
    for b in (4, 2, 1):
        lv.append([-b if (i & b) else b, 2])
    return lv


def build_program(b_pc=B_PC):
    nc = bacc.Bacc()
    x_d = nc.dram_tensor("x", [b_pc, 128], F32, kind="ExternalInput")
    out_d = nc.dram_tensor("out", [b_pc, 256], F32, kind="ExternalOutput")
    cd = {
        k: nc.dram_tensor(k, list(s), F32 if t == "f32" else BF16,
                          kind="ExternalInput")
        for k, (s, t) in CONST_SHAPES.items()
    }

    n_grp = b_pc // 512
    xv = x_d[:].rearrange("(g s p) f -> p g s f", s=4, p=128)
    ov = out_d[:].rearrange("(g s p) f -> p g s f", s=4, p=128)

    with tile.TileContext(nc) as tc:
        with (
            tc.tile_pool(name="consts", bufs=1) as consts,
            tc.tile_pool(name="io", bufs=3) as io,
            tc.tile_pool(name="work", bufs=2) as work,
            tc.tile_pool(name="ps", bufs=1, space="PSUM") as ps,
        ):
            C = {}
            for k, (s, t) in CONST_SHAPES.items():
                C[k] = consts.tile(list(s), F32 if t == "f32" else BF16,
                                   name=k, tag=k)
                nc.sync.dma_start(out=C[k], in_=cd[k][:])
            ident = consts.tile([128, 128], F32)
            make_identity(nc, ident)
            ident16 = consts.tile([128, 128], BF16)
            nc.vector.tensor_copy(ident16[:], ident[:])

            for g in range(n_grp):
                xq = io.tile([128, 4, 128], F32)
                nc.sync.dma_start(out=xq, in_=xv[:, g, :, :])
                outq = io.tile([128, 4, 256], F32)

                # ---- h = mvlinear1(x): per-subtile transpose + matmul ----
                h_ps = ps.tile([128, 4, 256], F32, bufs=1, tag="h_ps")
                for s in range(4):
                    xT_ps = ps.tile([128, 128], F32, bufs=1, tag="xT")
                    nc.tensor.transpose(xT_ps[:], xq[:, s, :], ident[:])
                    xT = work.tile([128, 128], F32, bufs=2, tag="xT_sb")
                    nc.scalar.activation(xT[:], xT_ps[:], AF.Copy)
                    nc.tensor.matmul(h_ps[:, s, :], lhsT=xT[:], rhs=C["W1big"][:],
                                     start=True, stop=True)
                nc.vector.tensor_tensor(
                    _ap(h_ps, 0, [[256, 4], [1, 32]]),
                    _ap(h_ps, 0, [[256, 4], [1, 32]]),
                    _ap(C["b1r"], 0, [[0, 4], [1, 32]]), ALU.add)
                h = work.tile([128, 4, 256], F32, bufs=2, tag="h")
                h2 = work.tile([128, 4, 256], F32, bufs=2, tag="h2")
                nc.scalar.activation(h[:], h_ps[:], AF.Copy)
                nc.scalar.activation(h2[:], h_ps[:], AF.Square)

                # ---- MVReLU gates, g-major [g*32+n], batched over subtiles
                invt = work.tile([128, 4, 128], F32, bufs=1, tag="invt")
                nc.gpsimd.tensor_copy(invt[:, :, 0:32], h[:, :, 0:32])
                nc.gpsimd.tensor_add(invt[:, :, 32:64], h2[:, :, 32:64], h2[:, :, 64:96])
                nc.gpsimd.tensor_add(invt[:, :, 32:64], invt[:, :, 32:64], h2[:, :, 128:160])
                nc.gpsimd.tensor_add(invt[:, :, 64:96], h2[:, :, 96:128], h2[:, :, 160:192])
                nc.gpsimd.tensor_add(invt[:, :, 64:96], invt[:, :, 64:96], h2[:, :, 192:224])
                nc.gpsimd.tensor_copy(invt[:, :, 96:128], h2[:, :, 224:256])
                gp = work.tile([128, 4, 128], F32, bufs=1, tag="gp")
                arl = _ap(C["arelur"], 0, [[0, 4], [1, 128]])
                brl = _ap(C["brelur"], 0, [[0, 4], [1, 128]])
                nc.gpsimd.tensor_tensor(gp[:], invt[:], arl, ALU.mult)
                nc.gpsimd.tensor_tensor(gp[:], gp[:], brl, ALU.add)
                nc.vector.tensor_scalar_max(gp[:], gp[:], 0.0)

                # gate-mul (blade-major j-runs), f32 out for hi/lo split
                hgF = work.tile([128, 4, 256], F32, bufs=1, tag="hgF")
                for grade, j0, ln in ((0, 0, 1), (1, 1, 2), (2, 3, 1),
                                      (1, 4, 1), (2, 5, 2), (3, 7, 1)):
                    o_ap = _ap(hgF, j0 * 32, [[256, 4], [32, ln], [1, 32]])
                    h_ap = _ap(h, j0 * 32, [[256, 4], [32, ln], [1, 32]])
                    g_ap = _ap(gp, grade * 32, [[128, 4], [0, ln], [1, 32]])
                    nc.vector.tensor_tensor(o_ap, h_ap, g_ap, ALU.mult)
                hg_hi = work.tile([128, 4, 256], BF16, bufs=2, tag="hg_hi")
                nc.vector.tensor_copy(hg_hi[:], hgF[:])
                hg_hiF = work.tile([128, 4, 256], F32, bufs=1, tag="hg_hiF")
                nc.scalar.activation(hg_hiF[:], hg_hi[:], AF.Copy)
                hg_lo = work.tile([128, 4, 256], BF16, bufs=2, tag="hg_lo")
                nc.vector.tensor_tensor(hg_lo[:], hgF[:], hg_hiF[:], ALU.subtract)

                # ---- transposes of hi/lo halves, then merged Wr|Wl matmuls
                hgT_ps = ps.tile([128, 4, 4, 128], BF16, bufs=1, tag="hgT")
                for s in range(4):
                    nc.tensor.transpose(hgT_ps[:, s, 0, :], hg_hi[:, s, 0:128], ident16[:])
                    nc.tensor.transpose(hgT_ps[:, s, 1, :], hg_hi[:, s, 128:256], ident16[:])
                    nc.tensor.transpose(hgT_ps[:, s, 2, :], hg_lo[:, s, 0:128], ident16[:])
                    nc.tensor.transpose(hgT_ps[:, s, 3, :], hg_lo[:, s, 128:256], ident16[:])
                hgTs = work.tile([128, 4, 4, 128], BF16, bufs=2, tag="hgTs")
                nc.scalar.activation(hgTs[:], hgT_ps[:], AF.Copy)

                xr = work.tile([128, 4, 256], F32, bufs=2, tag="xr")
                xr2 = work.tile([128, 4, 256], F32, bufs=2, tag="xr2")
                hl_sb = work.tile([128, 4, 256], BF16, bufs=2, tag="hl_sb")
                hf = work.tile([128, 4, 256], F32, bufs=2, tag="hf")
                geo = work.tile([128, 4, 256], BF16, bufs=2, tag="geo")
                for s in range(4):
                    xrhl_ps = ps.tile([128, 512], F32, bufs=2, tag="xrhl")
                    mms = [(0, "WWA_hi", True, False), (1, "WWB_hi", False, False),
                           (0, "WWA_lo", False, False), (1, "WWB_lo", False, False),
                           (2, "WWA_hi", False, False), (3, "WWB_hi", False, True)]
                    for piece, wname, st, sp in mms:
                        nc.tensor.matmul(xrhl_ps[:], lhsT=hgTs[:, s, piece, :],
                                         rhs=C[wname][:], start=st, stop=sp)
                    nc.scalar.activation(xr[:, s, :], xrhl_ps[:, 0:256], AF.Copy)
                    nc.scalar.activation(xr2[:, s, :], xrhl_ps[:, 0:256], AF.Square)
                    nc.scalar.activation(hl_sb[:, s, :], xrhl_ps[:, 256:512], AF.Copy)

                # ---- steerable norms (batched) ----
                qst = work.tile([128, 4, 128], F32, bufs=1, tag="qst")
                nc.gpsimd.tensor_copy(qst[:, :, 0:32], xr2[:, :, 0:32])
                nc.gpsimd.tensor_add(qst[:, :, 32:64], xr2[:, :, 32:64], xr2[:, :, 64:96])
                nc.gpsimd.tensor_add(qst[:, :, 32:64], qst[:, :, 32:64], xr2[:, :, 128:160])
                nc.gpsimd.tensor_add(qst[:, :, 64:96], xr2[:, :, 96:128], xr2[:, :, 160:192])
                nc.gpsimd.tensor_add(qst[:, :, 64:96], qst[:, :, 64:96], xr2[:, :, 192:224])
                nc.gpsimd.tensor_copy(qst[:, :, 96:128], xr2[:, :, 224:256])
                nt = work.tile([128, 4, 128], F32, bufs=1, tag="nt")
                nc.scalar.activation(nt[:], qst[:], AF.Sqrt)
                dent = work.tile([128, 4, 128], F32, bufs=1, tag="dent")
                sgr = _ap(C["signr"], 0, [[0, 4], [1, 128]])
                b2r = _ap(C["bias2r"], 0, [[0, 4], [1, 128]])
                nc.gpsimd.tensor_tensor(dent[:], nt[:], sgr, ALU.mult)
                nc.gpsimd.tensor_tensor(dent[:], dent[:], b2r, ALU.add)
                rden = work.tile([128, 4, 128], F32, bufs=2, tag="rden")
                rsc = work.tile([128, 4, 128], F32, bufs=1, tag="rsc")
                nc.vector.reciprocal_approx_accurate(rden[:], dent[:], rsc[:])
                xrn = work.tile([128, 4, 256], BF16, bufs=2, tag="xrn")
                for grade, j0, ln in ((0, 0, 1), (1, 1, 2), (2, 3, 1),
                                      (1, 4, 1), (2, 5, 2), (3, 7, 1)):
                    o_ap = _ap(xrn, j0 * 32, [[256, 4], [32, ln], [1, 32]])
                    x_ap = _ap(xr, j0 * 32, [[256, 4], [32, ln], [1, 32]])
                    r_ap = _ap(rden, grade * 32, [[128, 4], [0, ln], [1, 32]])
                    nc.vector.tensor_tensor(o_ap, x_ap, r_ap, ALU.mult)

                # ---- geometric product ----
                P2g = work.tile([128, 4, 2048], BF16, bufs=1, tag="P2g")
                for s in range(4):
                    V = work.tile([128, 2048], BF16, bufs=2, tag="V")
                    for i in range(8):
                        s2 = -64 if (i & 2) else 64
                        s1 = -32 if (i & 1) else 32
                        if (i & 3) in (0, 3):
                            s4 = -128 if (i & 4) else 128
                            out_ap = _ap(V, i * 256, [[128, 2], [32, 4], [1, 32]])
                            xr_ap = _ap(xrn, s * 256 + i * 32,
                                        [[s4, 2], [s1 // abs(s1) * 32, 4], [1, 32]])
                            w_ap = _ap(C["wrows"], i * 256,
                                       [[128, 2], [32, 4], [1, 32]])
                            nc.vector.tensor_tensor(out_ap, xr_ap, w_ap, ALU.mult)
                        else:
                            for j2 in (0, 1):
                                k0 = ((i ^ (j2 << 2)) & 4) * 32 + (i & 3) * 32
                                out_ap = _ap(V, i * 256 + j2 * 128,
                                             [[64, 2], [32, 2], [1, 32]])
                                xr_ap = _ap(xrn, s * 256 + k0,
                                            [[s2, 2], [s1, 2], [1, 32]])
                                w_ap = _ap(C["wrows"], i * 256 + j2 * 128,
                                           [[64, 2], [32, 2], [1, 32]])
                                nc.vector.tensor_tensor(out_ap, xr_ap, w_ap, ALU.mult)
                    hg_ap = _ap(hg_hi, s * 256, [[32, 8], [0, 8], [1, 32]])
                    v_ap = _ap(V, 0, [[256, 8], [32, 8], [1, 32]])
                    p2_ap = _ap(P2g, s * 2048, [[256, 8], [32, 8], [1, 32]])
                    nc.vector.tensor_tensor(p2_ap, hg_ap, v_ap, ALU.mult)
                # tree-reduce over i, in place, batched over subtiles
                nc.vector.tensor_tensor(
                    _ap(P2g, 0, [[2048, 4], [1, 1024]]),
                    _ap(P2g, 0, [[2048, 4], [1, 1024]]),
                    _ap(P2g, 1024, [[2048, 4], [1, 1024]]), ALU.add)
                nc.vector.tensor_tensor(
                    _ap(P2g, 0, [[2048, 4], [1, 512]]),
                    _ap(P2g, 0, [[2048, 4], [1, 512]]),
                    _ap(P2g, 512, [[2048, 4], [1, 512]]), ALU.add)
                nc.vector.tensor_tensor(
                    _ap(geo, 0, [[256, 4], [1, 256]]),
                    _ap(P2g, 0, [[2048, 4], [1, 256]]),
                    _ap(P2g, 256, [[2048, 4], [1, 256]]), ALU.add)

                nc.vector.tensor_tensor(
                    _ap(hl_sb, 0, [[256, 4], [1, 32]]),
                    _ap(hl_sb, 0, [[256, 4], [1, 32]]),
                    _ap(C["blr"], 0, [[0, 4], [1, 32]]), ALU.add)
                nc.vector.tensor_add(hf[:], hl_sb[:], geo[:])

                # ---- layernorm + output ----
                hf2 = work.tile([128, 4, 256], F32, bufs=2, tag="hf2")
                nc.scalar.activation(hf2[:], hf[:], AF.Square)
                u1 = work.tile([128, 4, 128], F32, bufs=1, tag="u1")
                nc.vector.tensor_add(u1[:], hf2[:, :, 0:128], hf2[:, :, 128:256])
                u2 = work.tile([128, 4, 64], F32, bufs=1, tag="u2")
                nc.vector.tensor_add(u2[:], u1[:, :, 0:64], u1[:, :, 64:128])
                s32 = work.tile([128, 4, 32], F32, bufs=1, tag="s32")
                nc.vector.tensor_add(s32[:], u2[:, :, 0:32], u2[:, :, 32:64])
                cn = work.tile([128, 4, 32], F32, bufs=1, tag="cn")
                nc.scalar.activation(cn[:], s32[:], AF.Sqrt)
                ivr = _ap(C["invalnr"], 0, [[0, 4], [1, 32]])
                nc.vector.tensor_tensor(cn[:], cn[:], ivr, ALU.mult)
                snrm = work.tile([128, 4], F32, bufs=1, tag="snrm")
                nc.vector.tensor_reduce(snrm[:].unsqueeze(2), cn[:],
                                        axis=AX.X, op=ALU.add)
                den = work.tile([128, 4], F32, bufs=1, tag="den")
                nc.vector.tensor_scalar(den[:], snrm[:], 1.0 / 32.0, EPS,
                                        op0=ALU.mult, op1=ALU.add)
                rr = work.tile([128, 4], F32, bufs=2, tag="rr")
                nc.vector.reciprocal(rr[:], den[:])
                for s in range(4):
                    nc.scalar.activation(outq[:, s, :], hf[:, s, :], AF.Identity,
                                         scale=rr[:, s:s + 1])

                nc.sync.dma_start(out=ov[:, g, :, :], in_=outq)
    nc.finalize()
    return nc


_PROG = {}
LAST_RESULT = None


def _get_program(b_pc):
    if b_pc not in _PROG:
        _PROG[b_pc] = build_program(b_pc)
    return _PROG[b_pc]


def kernel(**inputs):
    x = np.ascontiguousarray(np.asarray(inputs["x"], np.float32))
    consts = build_consts(
        np.asarray(inputs["w1"], np.float32), np.asarray(inputs["b1"], np.float32),
        np.asarray(inputs["a_relu"], np.float32), np.asarray(inputs["b_relu"], np.float32),
        np.asarray(inputs["wl"], np.float32), np.asarray(inputs["bl"], np.float32),
        np.asarray(inputs["wr"], np.float32), np.asarray(inputs["a_norm"], np.float32),
        np.asarray(inputs["gp_w"], np.float32), np.asarray(inputs["a_ln"], np.float32),
    )
    import ml_dtypes
    for k, (_, t) in CONST_SHAPES.items():
        if t == "bf16":
            consts[k] = consts[k].astype(ml_dtypes.bfloat16)
    b_total = x.shape[0]
    b_pc = b_total // N_CORES
    nc = _get_program(b_pc)
    in_maps = []
    for c in range(N_CORES):
        m = {"x": x[c * b_pc:(c + 1) * b_pc].reshape(b_pc, 128)}
        m.update(consts)
        in_maps.append(m)
    import os
    trace = os.environ.get("KERNEL_TRACE", "0") == "1"
    res = run_bass_kernel_spmd(nc, in_maps, core_ids=list(range(N_CORES)),
                               trace=trace)
    global LAST_RESULT
    LAST_RESULT = res
    outs = [
        res.results[c]["out"].reshape(b_pc, 8, FOUT)[:, MASKS, :].transpose(0, 2, 1)
        for c in range(N_CORES)
    ]
    return np.ascontiguousarray(np.concatenate(outs, axis=0).astype(np.float32))


if __name__ == "__main__":
    # smoke test with random data against a numpy re-implementation
    rng = np.random.default_rng(0)
    print("building program...")
    build_program(512)
    print("ok")


# revision 28
# speedup vs baseline: 1.4683x; 1.0145x over previous
"""Trainium2 Bass kernel for nn_CGEBlock (Clifford Group Equivariant block, Cl(3,0)).

Strategy: pure data-parallel over the batch dim (8 cores x 16384 points).
Internally blades are kept in *mask order* (blade index == bitmask of basis
vectors), which makes the geometric-product pairing (i, k=i^j) expressible as
nested +-stride access patterns on the Vector engine. The final result is
permuted back to the reference blade order on the host.

Layout on device: "natural" — batch on partitions (tiles of 128 points),
features*blades on the free dim. Matmuls run on PE with the activation tile
transposed via the tensor engine; gates/norms use ACT; products use DVE.
"""

import sys

for p in ("/opt/trn_rl_repo",):
    if p not in sys.path:
        sys.path.insert(0, p)

import numpy as np

import concourse.bass as bass
import concourse.bacc as bacc
import concourse.mybir as mybir
import concourse.tile as tile
from concourse.bass_utils import run_bass_kernel_spmd
from concourse.masks import make_identity

EPS = 1e-6
N_CORES = 8
B_TOTAL = 131072
B_PC = B_TOTAL // N_CORES  # 16384
FIN = 16
FOUT = 32

# blade index (reference order) -> bitmask; also its own inverse permutation
MASKS = [0, 1, 2, 4, 3, 5, 6, 7]
GRADE_IDX = [0, 1, 1, 1, 2, 2, 2, 3]
PC = [bin(m).count("1") for m in range(8)]  # grade of a mask
# mask positions per grade (for strided slicing of mask-ordered 8-blocks)
GPOS = {0: [0], 1: [1, 2, 4], 2: [3, 5, 6], 3: [7]}

F32 = mybir.dt.float32
BF16 = mybir.dt.bfloat16
AX = mybir.AxisListType
ALU = mybir.AluOpType
AF = mybir.ActivationFunctionType

# contiguous mask-position runs sharing one grade: (grade, [positions])
GRUNS = [(0, 0, 1), (1, 1, 2), (2, 3, 1), (1, 4, 1), (2, 5, 2), (3, 7, 1)]


def _cayley_sign(a, b):
    s, aa = 0, a >> 1
    while aa:
        s += bin(aa & b).count("1")
        aa >>= 1
    return -1.0 if (s & 1) else 1.0


def build_consts(w1, b1, a_relu, b_relu, wl, bl, wr, a_norm, gp_w, a_ln):
    """Host-side constant matrices.

    Feature axes use *blade-major* layout: column index = jm*32 + n, where jm
    is the blade bitmask and n the channel. This keeps every Vector-engine op
    contiguous over 32-channel runs (strided access patterns run ~5x slower).
    """
    c = {}
    isq2 = 1.0 / np.sqrt(2.0)

    # W1big [128=(m,i_idx), 256=(jm,n)] : h = x @ W1big (+b1 on mask-0 blade)
    W1 = np.zeros((128, 256), np.float32)
    for m in range(FIN):
        for ii in range(8):
            jm = MASKS[ii]
            for n in range(FOUT):
                W1[m * 8 + ii, jm * 32 + n] = w1[n, m, GRADE_IDX[ii]]
    c["W1big"] = W1

    # Wr/Wl halves: rows (jm,n) blade-major halves, cols (jm',n') blade-major
    WrA = np.zeros((128, 256), np.float32)
    WrB = np.zeros((128, 256), np.float32)
    WlA = np.zeros((128, 256), np.float32)
    WlB = np.zeros((128, 256), np.float32)
    for jm in range(8):
        g = PC[jm]
        half, base = (WrA, jm * 32) if jm < 4 else (WrB, (jm - 4) * 32)
        halfl = WlA if jm < 4 else WlB
        for n in range(FOUT):
            for n2 in range(FOUT):
                half[base + n, jm * 32 + n2] = wr[n2, n, g]
                halfl[base + n, jm * 32 + n2] = wl[n2, n, g] * a_ln[n2] * isq2
    # merge Wr|Wl per K-half and split hi/lo in bf16 (removes weight rounding)
    import ml_dtypes
    for nm, a, b in (("A", WrA, WlA), ("B", WrB, WlB)):
        M = np.concatenate([a, b], axis=1)
        hi = M.astype(ml_dtypes.bfloat16).astype(np.float32)
        lo = M - hi
        c[f"WW{nm}_hi"] = hi
        c[f"WW{nm}_lo"] = lo

    rep = lambda v: np.repeat(v[None, :].astype(np.float32), 128, 0)
    c["b1r"] = rep(b1)
    c["blr"] = rep(bl * a_ln * isq2)
    c["invalnr"] = rep(1.0 / a_ln)

    # gate / norm rows, g-major layout: col = g*32 + n
    c["arelur"] = rep(a_relu.T.reshape(-1))
    c["brelur"] = rep(b_relu.T.reshape(-1))
    sig = 1.0 / (1.0 + np.exp(-a_norm))
    c["signr"] = rep(sig.T.reshape(-1))
    c["bias2r"] = rep((1.0 - sig + EPS).T.reshape(-1))

    # wrowsP [128, 8*256], layout (i, jm, n): coeff for h-blade i at (jm, n)
    W = np.zeros((8, 256), np.float32)
    for i in range(8):
        for jm in range(8):
            s = _cayley_sign(i, jm)
            gw = gp_w[:, PC[i], PC[jm], PC[i ^ jm]]  # [C]
            for cn in range(FOUT):
                W[i, jm * 32 + cn] = s * gw[cn] * a_ln[cn] * isq2
    c["wrows"] = np.repeat(W.reshape(1, -1), 128, 0).astype(np.float32)
    return c


CONST_SHAPES = {
    "W1big": ((128, 256), "f32"),
    "WWA_hi": ((128, 512), "bf16"),
    "WWA_lo": ((128, 512), "bf16"),
    "WWB_hi": ((128, 512), "bf16"),
    "WWB_lo": ((128, 512), "bf16"),
    "b1r": ((128, 32), "f32"),
    "blr": ((128, 32), "f32"),
    "invalnr": ((128, 32), "f32"),
    "arelur": ((128, 128), "f32"),
    "brelur": ((128, 128), "f32"),
    "signr": ((128, 128), "f32"),
    "bias2r": ((128, 128), "f32"),
    "wrows": ((128, 2048), "bf16"),
}


def _ap(t, off, levels):
    """Custom free-dim AP on tile t: keep partition level, replace free levels."""
    a = t[:]
    return bass.AP(tensor=a.tensor, offset=a.offset + off, ap=[list(a.ap[0])] + levels)


def _xor_levels(i, cstep=8):
    """Nested levels reading index c*cstep + (i ^ j) as (c, j2, j1, j0)."""
    lv = [[cstep, 32]]
    for b in (4, 2, 1):
        lv.append([-b if (i & b) else b, 2])
    return lv


def build_program(b_pc=B_PC):
    nc = bacc.Bacc()
    x_d = nc.dram_tensor("x", [b_pc, 128], F32, kind="ExternalInput")
    out_d = nc.dram_tensor("out", [b_pc, 256], F32, kind="ExternalOutput")
    cd = {
        k: nc.dram_tensor(k, list(s), F32 if t == "f32" else BF16,
                          kind="ExternalInput")
        for k, (s, t) in CONST_SHAPES.items()
    }

    n_grp = b_pc // 512
    xv = x_d[:].rearrange("(g s p) f -> p g s f", s=4, p=128)
    ov = out_d[:].rearrange("(g s p) f -> p g s f", s=4, p=128)

    with tile.TileContext(nc) as tc:
        with (
            tc.tile_pool(name="consts", bufs=1) as consts,
            tc.tile_pool(name="io", bufs=3) as io,
            tc.tile_pool(name="work", bufs=2) as work,
            tc.tile_pool(name="ps", bufs=1, space="PSUM") as ps,
        ):
            C = {}
            for k, (s, t) in CONST_SHAPES.items():
                C[k] = consts.tile(list(s), F32 if t == "f32" else BF16,
                                   name=k, tag=k)
                nc.sync.dma_start(out=C[k], in_=cd[k][:])
            ident = consts.tile([128, 128], F32)
            make_identity(nc, ident)
            ident16 = consts.tile([128, 128], BF16)
            nc.vector.tensor_copy(ident16[:], ident[:])

            for g in range(n_grp):
                xq = io.tile([128, 4, 128], F32)
                nc.sync.dma_start(out=xq, in_=xv[:, g, :, :])
                outq = io.tile([128, 4, 256], F32)

                # ---- h = mvlinear1(x): per-subtile transpose + matmul ----
                h_ps = ps.tile([128, 4, 256], F32, bufs=1, tag="h_ps")
                for s in range(4):
                    xT_ps = ps.tile([128, 128], F32, bufs=2, tag="xT")
                    nc.tensor.transpose(xT_ps[:], xq[:, s, :], ident[:])
                    xT = work.tile([128, 128], F32, bufs=2, tag="xT_sb")
                    nc.scalar.activation(xT[:], xT_ps[:], AF.Copy)
                    nc.tensor.matmul(h_ps[:, s, :], lhsT=xT[:], rhs=C["W1big"][:],
                                     start=True, stop=True)
                nc.vector.tensor_tensor(
                    _ap(h_ps, 0, [[256, 4], [1, 32]]),
                    _ap(h_ps, 0, [[256, 4], [1, 32]]),
                    _ap(C["b1r"], 0, [[0, 4], [1, 32]]), ALU.add)
                h = work.tile([128, 4, 256], F32, bufs=2, tag="h")
                h2 = work.tile([128, 4, 256], F32, bufs=2, tag="h2")
                nc.scalar.activation(h[:], h_ps[:], AF.Copy)
                nc.scalar.activation(h2[:], h_ps[:], AF.Square)

                # ---- MVReLU gates, g-major [g*32+n], batched over subtiles
                invt = work.tile([128, 4, 128], F32, bufs=2, tag="invt")
                nc.gpsimd.tensor_copy(invt[:, :, 0:32], h[:, :, 0:32])
                nc.gpsimd.tensor_add(invt[:, :, 32:64], h2[:, :, 32:64], h2[:, :, 64:96])
                nc.gpsimd.tensor_add(invt[:, :, 32:64], invt[:, :, 32:64], h2[:, :, 128:160])
                nc.gpsimd.tensor_add(invt[:, :, 64:96], h2[:, :, 96:128], h2[:, :, 160:192])
                nc.gpsimd.tensor_add(invt[:, :, 64:96], invt[:, :, 64:96], h2[:, :, 192:224])
                nc.gpsimd.tensor_copy(invt[:, :, 96:128], h2[:, :, 224:256])
                gp = work.tile([128, 4, 128], F32, bufs=2, tag="gp")
                arl = _ap(C["arelur"], 0, [[0, 4], [1, 128]])
                brl = _ap(C["brelur"], 0, [[0, 4], [1, 128]])
                nc.gpsimd.tensor_tensor(gp[:], invt[:], arl, ALU.mult)
                nc.gpsimd.tensor_tensor(gp[:], gp[:], brl, ALU.add)
                nc.vector.tensor_scalar_max(gp[:], gp[:], 0.0)

                # gate-mul (blade-major j-runs), f32 out for hi/lo split
                hgF = work.tile([128, 4, 256], F32, bufs=2, tag="hgF")
                for grade, j0, ln in ((0, 0, 1), (1, 1, 2), (2, 3, 1),
                                      (1, 4, 1), (2, 5, 2), (3, 7, 1)):
                    o_ap = _ap(hgF, j0 * 32, [[256, 4], [32, ln], [1, 32]])
                    h_ap = _ap(h, j0 * 32, [[256, 4], [32, ln], [1, 32]])
                    g_ap = _ap(gp, grade * 32, [[128, 4], [0, ln], [1, 32]])
                    nc.vector.tensor_tensor(o_ap, h_ap, g_ap, ALU.mult)
                hg_hi = work.tile([128, 4, 256], BF16, bufs=2, tag="hg_hi")
                nc.vector.tensor_copy(hg_hi[:], hgF[:])
                hg_hiF = work.tile([128, 4, 256], F32, bufs=2, tag="hg_hiF")
                nc.scalar.activation(hg_hiF[:], hg_hi[:], AF.Copy)
                hg_lo = work.tile([128, 4, 256], BF16, bufs=2, tag="hg_lo")
                nc.vector.tensor_tensor(hg_lo[:], hgF[:], hg_hiF[:], ALU.subtract)

                # ---- transposes of hi/lo halves, then merged Wr|Wl matmuls
                hgT_ps = ps.tile([128, 4, 4, 128], BF16, bufs=1, tag="hgT")
                for s in range(4):
                    nc.tensor.transpose(hgT_ps[:, s, 0, :], hg_hi[:, s, 0:128], ident16[:])
                    nc.tensor.transpose(hgT_ps[:, s, 1, :], hg_hi[:, s, 128:256], ident16[:])
                    nc.tensor.transpose(hgT_ps[:, s, 2, :], hg_lo[:, s, 0:128], ident16[:])
                    nc.tensor.transpose(hgT_ps[:, s, 3, :], hg_lo[:, s, 128:256], ident16[:])
                hgTs = work.tile([128, 4, 4, 128], BF16, bufs=2, tag="hgTs")
                nc.scalar.activation(hgTs[:], hgT_ps[:], AF.Copy)

                xr = work.tile([128, 4, 256], F32, bufs=2, tag="xr")
                xr2 = work.tile([128, 4, 256], F32, bufs=2, tag="xr2")
                hl_sb = work.tile([128, 4, 256], BF16, bufs=2, tag="hl_sb")
                hf = work.tile([128, 4, 256], F32, bufs=2, tag="hf")
                geo = work.tile([128, 4, 256], BF16, bufs=2, tag="geo")
                for s in range(4):
                    xrhl_ps = ps.tile([128, 512], F32, bufs=2, tag="xrhl")
                    mms = [(0, "WWA_hi", True, False), (1, "WWB_hi", False, False),
                           (0, "WWA_lo", False, False), (1, "WWB_lo", False, False),
                           (2, "WWA_hi", False, False), (3, "WWB_hi", False, True)]
                    for piece, wname, st, sp in mms:
                        nc.tensor.matmul(xrhl_ps[:], lhsT=hgTs[:, s, piece, :],
                                         rhs=C[wname][:], start=st, stop=sp)
                    nc.scalar.activation(xr[:, s, :], xrhl_ps[:, 0:256], AF.Copy)
                    nc.scalar.activation(xr2[:, s, :], xrhl_ps[:, 0:256], AF.Square)
                    nc.scalar.activation(hl_sb[:, s, :], xrhl_ps[:, 256:512], AF.Copy)

                # ---- steerable norms (batched) ----
                qst = work.tile([128, 4, 128], F32, bufs=2, tag="qst")
                nc.gpsimd.tensor_copy(qst[:, :, 0:32], xr2[:, :, 0:32])
                nc.gpsimd.tensor_add(qst[:, :, 32:64], xr2[:, :, 32:64], xr2[:, :, 64:96])
                nc.gpsimd.tensor_add(qst[:, :, 32:64], qst[:, :, 32:64], xr2[:, :, 128:160])
                nc.gpsimd.tensor_add(qst[:, :, 64:96], xr2[:, :, 96:128], xr2[:, :, 160:192])
                nc.gpsimd.tensor_add(qst[:, :, 64:96], qst[:, :, 64:96], xr2[:, :, 192:224])
                nc.gpsimd.tensor_copy(qst[:, :, 96:128], xr2[:, :, 224:256])
                nt = work.tile([128, 4, 128], F32, bufs=2, tag="nt")
                nc.scalar.activation(nt[:], qst[:], AF.Sqrt)
                dent = work.tile([128, 4, 128], F32, bufs=2, tag="dent")
                sgr = _ap(C["signr"], 0, [[0, 4], [1, 128]])
                b2r = _ap(C["bias2r"], 0, [[0, 4], [1, 128]])
                nc.gpsimd.tensor_tensor(dent[:], nt[:], sgr, ALU.mult)
                nc.gpsimd.tensor_tensor(dent[:], dent[:], b2r, ALU.add)
                rden = work.tile([128, 4, 128], F32, bufs=2, tag="rden")
                rsc = work.tile([128, 4, 128], F32, bufs=2, tag="rsc")
                nc.vector.reciprocal_approx_accurate(rden[:], dent[:], rsc[:])
                xrn = work.tile([128, 4, 256], BF16, bufs=2, tag="xrn")
                for grade, j0, ln in ((0, 0, 1), (1, 1, 2), (2, 3, 1),
                                      (1, 4, 1), (2, 5, 2), (3, 7, 1)):
                    o_ap = _ap(xrn, j0 * 32, [[256, 4], [32, ln], [1, 32]])
                    x_ap = _ap(xr, j0 * 32, [[256, 4], [32, ln], [1, 32]])
                    r_ap = _ap(rden, grade * 32, [[128, 4], [0, ln], [1, 32]])
                    nc.vector.tensor_tensor(o_ap, x_ap, r_ap, ALU.mult)

                # ---- geometric product ----
                P2g = work.tile([128, 4, 2048], BF16, bufs=2, tag="P2g")
                for s in range(4):
                    V = work.tile([128, 2048], BF16, bufs=2, tag="V")
                    for i in range(8):
                        s2 = -64 if (i & 2) else 64
                        s1 = -32 if (i & 1) else 32
                        if (i & 3) in (0, 3):
                            s4 = -128 if (i & 4) else 128
                            out_ap = _ap(V, i * 256, [[128, 2], [32, 4], [1, 32]])
                            xr_ap = _ap(xrn, s * 256 + i * 32,
                                        [[s4, 2], [s1 // abs(s1) * 32, 4], [1, 32]])
                            w_ap = _ap(C["wrows"], i * 256,
                                       [[128, 2], [32, 4], [1, 32]])
                            nc.vector.tensor_tensor(out_ap, xr_ap, w_ap, ALU.mult)
                        else:
                            for j2 in (0, 1):
                                k0 = ((i ^ (j2 << 2)) & 4) * 32 + (i & 3) * 32
                                out_ap = _ap(V, i * 256 + j2 * 128,
                                             [[64, 2], [32, 2], [1, 32]])
                                xr_ap = _ap(xrn, s * 256 + k0,
                                            [[s2, 2], [s1, 2], [1, 32]])
                                w_ap = _ap(C["wrows"], i * 256 + j2 * 128,
                                           [[64, 2], [32, 2], [1, 32]])
                                nc.vector.tensor_tensor(out_ap, xr_ap, w_ap, ALU.mult)
                    hg_ap = _ap(hg_hi, s * 256, [[32, 8], [0, 8], [1, 32]])
                    v_ap = _ap(V, 0, [[256, 8], [32, 8], [1, 32]])
                    p2_ap = _ap(P2g, s * 2048, [[256, 8], [32, 8], [1, 32]])
                    nc.vector.tensor_tensor(p2_ap, hg_ap, v_ap, ALU.mult)
                # tree-reduce over i, in place, batched over subtiles
                nc.vector.tensor_tensor(
                    _ap(P2g, 0, [[2048, 4], [1, 1024]]),
                    _ap(P2g, 0, [[2048, 4], [1, 1024]]),
                    _ap(P2g, 1024, [[2048, 4], [1, 1024]]), ALU.add)
                nc.vector.tensor_tensor(
                    _ap(P2g, 0, [[2048, 4], [1, 512]]),
                    _ap(P2g, 0, [[2048, 4], [1, 512]]),
                    _ap(P2g, 512, [[2048, 4], [1, 512]]), ALU.add)
                nc.vector.tensor_tensor(
                    _ap(geo, 0, [[256, 4], [1, 256]]),
                    _ap(P2g, 0, [[2048, 4], [1, 256]]),
                    _ap(P2g, 256, [[2048, 4], [1, 256]]), ALU.add)

                nc.vector.tensor_tensor(
                    _ap(hl_sb, 0, [[256, 4], [1, 32]]),
                    _ap(hl_sb, 0, [[256, 4], [1, 32]]),
                    _ap(C["blr"], 0, [[0, 4], [1, 32]]), ALU.add)
                nc.vector.tensor_add(hf[:], hl_sb[:], geo[:])

                # ---- layernorm + output ----
                hf2 = work.tile([128, 4, 256], F32, bufs=2, tag="hf2")
                nc.scalar.activation(hf2[:], hf[:], AF.Square)
                u1 = work.tile([128, 4, 128], F32, bufs=2, tag="u1")
                nc.vector.tensor_add(u1[:], hf2[:, :, 0:128], hf2[:, :, 128:256])
                u2 = work.tile([128, 4, 64], F32, bufs=2, tag="u2")
                nc.vector.tensor_add(u2[:], u1[:, :, 0:64], u1[:, :, 64:128])
                s32 = work.tile([128, 4, 32], F32, bufs=2, tag="s32")
                nc.vector.tensor_add(s32[:], u2[:, :, 0:32], u2[:, :, 32:64])
                cn = work.tile([128, 4, 32], F32, bufs=2, tag="cn")
                nc.scalar.activation(cn[:], s32[:], AF.Sqrt)
                ivr = _ap(C["invalnr"], 0, [[0, 4], [1, 32]])
                nc.vector.tensor_tensor(cn[:], cn[:], ivr, ALU.mult)
                snrm = work.tile([128, 4], F32, bufs=2, tag="snrm")
                nc.vector.tensor_reduce(snrm[:].unsqueeze(2), cn[:],
                                        axis=AX.X, op=ALU.add)
                den = work.tile([128, 4], F32, bufs=2, tag="den")
                nc.vector.tensor_scalar(den[:], snrm[:], 1.0 / 32.0, EPS,
                                        op0=ALU.mult, op1=ALU.add)
                rr = work.tile([128, 4], F32, bufs=2, tag="rr")
                nc.vector.reciprocal(rr[:], den[:])
                for s in range(4):
                    nc.scalar.activation(outq[:, s, :], hf[:, s, :], AF.Identity,
                                         scale=rr[:, s:s + 1])

                nc.sync.dma_start(out=ov[:, g, :, :], in_=outq)
    nc.finalize()
    return nc


_PROG = {}
LAST_RESULT = None


def _get_program(b_pc):
    if b_pc not in _PROG:
        _PROG[b_pc] = build_program(b_pc)
    return _PROG[b_pc]


def kernel(**inputs):
    x = np.ascontiguousarray(np.asarray(inputs["x"], np.float32))
    consts = build_consts(
        np.asarray(inputs["w1"], np.float32), np.asarray(inputs["b1"], np.float32),
        np.asarray(inputs["a_relu"], np.float32), np.asarray(inputs["b_relu"], np.float32),
        np.asarray(inputs["wl"], np.float32), np.asarray(inputs["bl"], np.float32),
        np.asarray(inputs["wr"], np.float32), np.asarray(inputs["a_norm"], np.float32),
        np.asarray(inputs["gp_w"], np.float32), np.asarray(inputs["a_ln"], np.float32),
    )
    import ml_dtypes
    for k, (_, t) in CONST_SHAPES.items():
        if t == "bf16":
            consts[k] = consts[k].astype(ml_dtypes.bfloat16)
    b_total = x.shape[0]
    b_pc = b_total // N_CORES
    nc = _get_program(b_pc)
    in_maps = []
    for c in range(N_CORES):
        m = {"x": x[c * b_pc:(c + 1) * b_pc].reshape(b_pc, 128)}
        m.update(consts)
        in_maps.append(m)
    import os
    trace = os.environ.get("KERNEL_TRACE", "0") == "1"
    res = run_bass_kernel_spmd(nc, in_maps, core_ids=list(range(N_CORES)),
                               trace=trace)
    global LAST_RESULT
    LAST_RESULT = res
    outs = [
        res.results[c]["out"].reshape(b_pc, 8, FOUT)[:, MASKS, :].transpose(0, 2, 1)
        for c in range(N_CORES)
    ]
    return np.ascontiguousarray(np.concatenate(outs, axis=0).astype(np.float32))


if __name__ == "__main__":
    # smoke test with random data against a numpy re-implementation
    rng = np.random.default_rng(0)
    print("building program...")
    build_program(512)
    print("ok")
